# revision 9
# baseline (speedup 1.0000x reference)
"""Distributed multi-head attention (RoPE + SDPA + out-proj) for one TRN2 chip.

Sharding: 8 cores = 4 batches x 2 query-halves. Each core computes K/V for the
full sequence of its batch (duplicated with its pair core -> zero collectives)
and Q/attention/out-proj for its 1024-query half. Host gathers by pure
concatenation.

Layouts (all chosen so every matmul operand is contiguous in SBUF):
  - projections produce transposed activations: qT/kT [o, t], v [t, o]
  - scores computed transposed: sT[k, q] = KT_h.T @ QT_h   (contraction dh=64)
  - softmax denominator via a ones-column appended to V (psum row 64)
  - PV: ctxT[dh, q] = V_h.T-free matmul, normalize with PE-broadcast recip
  - out[t, o] = ctxT.T @ woT
Compute dtype bf16 (fp32 PSUM accumulation); measured end-to-end rel err ~5e-3.
"""

from contextlib import ExitStack

import ml_dtypes
import numpy as np

import concourse.bass as bass
import concourse.tile as tile
from concourse import bacc, bass_utils, mybir

B, S, D, H, DH = 4, 2048, 1024, 16, 64
SQ = S // 2          # queries per core
NCORES = 8
BF = mybir.dt.bfloat16
F32 = mybir.dt.float32
BF_NP = ml_dtypes.bfloat16

KT_D = D // 128      # 8  d-tiles (contraction for projections)
KT_S = S // 128      # 16 seq k-tiles
KT_SQ = SQ // 128    # 8  out t-tiles
NQ = SQ // 512       # 2  q chunks
NS = S // 512        # 4  seq chunks
NO = D // 512        # 2  o chunks


def _build():
    nc = bacc.Bacc("TRN2", target_bir_lowering=False, debug=False,
                   num_devices=NCORES)

    xT = nc.dram_tensor("xT", [D, S], BF, kind="ExternalInput").ap()
    xqT = nc.dram_tensor("xqT", [D, SQ], BF, kind="ExternalInput").ap()
    wqT = nc.dram_tensor("wqT", [D, D], BF, kind="ExternalInput").ap()
    wkT = nc.dram_tensor("wkT", [D, D], BF, kind="ExternalInput").ap()
    wvT = nc.dram_tensor("wvT", [D, D], BF, kind="ExternalInput").ap()
    woT = nc.dram_tensor("woT", [D, D], BF, kind="ExternalInput").ap()
    cosq = nc.dram_tensor("cosq", [128, SQ], BF, kind="ExternalInput").ap()
    sinrq = nc.dram_tensor("sinrq", [128, SQ], BF, kind="ExternalInput").ap()
    cosk = nc.dram_tensor("cosk", [128, S], BF, kind="ExternalInput").ap()
    sinrk = nc.dram_tensor("sinrk", [128, S], BF, kind="ExternalInput").ap()
    out = nc.dram_tensor("out", [SQ, D], F32, kind="ExternalOutput").ap()

    with tile.TileContext(nc) as tc, ExitStack() as ctx:
        # ---- persistent pools (live across phases) ----
        qr_pool = ctx.enter_context(tc.tile_pool(name="qr", bufs=KT_D))
        kr_pool = ctx.enter_context(tc.tile_pool(name="kr", bufs=KT_D))
        v_pool = ctx.enter_context(tc.tile_pool(name="v", bufs=KT_S))
        ctxT_pool = ctx.enter_context(tc.tile_pool(name="ctxT", bufs=KT_D))
        wo_pool = ctx.enter_context(tc.tile_pool(name="wo", bufs=KT_D))
        ones_pool = ctx.enter_context(tc.tile_pool(name="ones", bufs=1))

        qr_t = [qr_pool.tile([128, SQ], BF, tag="qr", name="qr") for _ in range(KT_D)]
        kr_t = [kr_pool.tile([128, S], BF, tag="kr", name="kr") for _ in range(KT_D)]
        v_t = [v_pool.tile([128, H * (DH + 1)], BF, tag="v", name="v")
               for _ in range(KT_S)]
        ctxT_t = [ctxT_pool.tile([128, SQ], BF, tag="ctxT", name="ctxT")
                  for _ in range(KT_D)]
        wo_t = [wo_pool.tile([128, D], BF, tag="wo", name="wo") for _ in range(KT_D)]
        ones_t = ones_pool.tile([128, DH], BF, tag="ones", name="ones")

        nc.vector.memset(ones_t[:], 1.0)
        for i in range(KT_D):
            nc.sync.dma_start(wo_t[i][:], woT[i * 128:(i + 1) * 128, :])
        # ones columns of v tiles (data columns overwritten by eviction)
        for i in range(KT_S):
            nc.vector.memset(v_t[i][:], 1.0)

        # ================= Phase A: projections + RoPE =================
        with ExitStack() as actx:
            x_pool = actx.enter_context(tc.tile_pool(name="x", bufs=KT_D))
            xq_pool = actx.enter_context(tc.tile_pool(name="xq", bufs=KT_D))
            w_pool = actx.enter_context(tc.tile_pool(name="w", bufs=KT_D))
            raw_pool = actx.enter_context(tc.tile_pool(name="raw", bufs=2))
            rot_pool = actx.enter_context(tc.tile_pool(name="rot", bufs=2))
            tab_pool = actx.enter_context(tc.tile_pool(name="tab", bufs=1))
            psA = actx.enter_context(
                tc.tile_pool(name="psA", bufs=4, space="PSUM"))

            x_t = [x_pool.tile([128, S], BF, tag="x", name="x") for _ in range(KT_D)]
            xq_t = [xq_pool.tile([128, SQ], BF, tag="xq", name="xq")
                    for _ in range(KT_D)]
            for i in range(KT_D):
                nc.sync.dma_start(x_t[i][:], xT[i * 128:(i + 1) * 128, :])
                nc.sync.dma_start(xq_t[i][:], xqT[i * 128:(i + 1) * 128, :])
            cosq_t = tab_pool.tile([128, SQ], BF, tag="cq")
            sinq_t = tab_pool.tile([128, SQ], BF, tag="sq")
            cosk_t = tab_pool.tile([128, S], BF, tag="ck")
            sink_t = tab_pool.tile([128, S], BF, tag="sk")
            nc.sync.dma_start(cosq_t[:], cosq[:])
            nc.sync.dma_start(sinq_t[:], sinrq[:])
            nc.sync.dma_start(cosk_t[:], cosk[:])
            nc.sync.dma_start(sink_t[:], sinrk[:])

            # ---- V projection: v[t, o] = x.T @ wvT, strided into 65-col blocks
            wv_t = [w_pool.tile([128, D], BF, tag="w", name="w") for _ in range(KT_D)]
            for i in range(KT_D):
                nc.sync.dma_start(wv_t[i][:], wvT[i * 128:(i + 1) * 128, :])
            for m in range(KT_S):
                for n in range(NO):
                    ps = psA.tile([128, 512], F32, tag="psA")
                    for k in range(KT_D):
                        nc.tensor.matmul(
                            ps[:], x_t[k][:, m * 128:(m + 1) * 128],
                            wv_t[k][:, n * 512:(n + 1) * 512],
                            start=(k == 0), stop=(k == KT_D - 1))
                    dst = v_t[m][:].rearrange("p (h c) -> p h c", c=DH + 1)
                    src = ps[:].rearrange("p (h c) -> p h c", c=DH)
                    nc.vector.tensor_copy(
                        dst[:, n * 8:(n + 1) * 8, 0:DH], src[:])

            def project_rope(w_dram, x_tiles, nt, nchunks, cos_t, sin_t,
                             out_tiles):
                w_t = [w_pool.tile([128, D], BF, tag="w", name="w")
                       for _ in range(KT_D)]
                for i in range(KT_D):
                    nc.sync.dma_start(w_t[i][:],
                                      w_dram[i * 128:(i + 1) * 128, :])
                T = nt  # free length of a row tile
                for m in range(KT_D):  # output o-tile
                    raw = raw_pool.tile([128, S], BF, tag="raw")
                    for n in range(nchunks):
                        ps = psA.tile([128, 512], F32, tag="psA")
                        for k in range(KT_D):
                            nc.tensor.matmul(
                                ps[:], w_t[k][:, m * 128:(m + 1) * 128],
                                x_tiles[k][:, n * 512:(n + 1) * 512],
                                start=(k == 0), stop=(k == KT_D - 1))
                        nc.scalar.activation(
                            raw[:, n * 512:(n + 1) * 512], ps[:],
                            mybir.ActivationFunctionType.Copy)
                    rot = rot_pool.tile([128, S], BF, tag="rot")
                    for b0 in (0, 64):
                        nc.vector.tensor_copy(rot[b0:b0 + 32, 0:T],
                                              raw[b0 + 32:b0 + 64, 0:T])
                        nc.vector.tensor_copy(rot[b0 + 32:b0 + 64, 0:T],
                                              raw[b0:b0 + 32, 0:T])
                    o = out_tiles[m]
                    nc.vector.tensor_mul(o[:, 0:T], raw[:, 0:T],
                                         cos_t[:, 0:T])
                    nc.vector.tensor_mul(rot[:, 0:T], rot[:, 0:T],
                                         sin_t[:, 0:T])
                    nc.vector.tensor_add(o[:, 0:T], o[:, 0:T], rot[:, 0:T])

            project_rope(wkT, x_t, S, NS, cosk_t, sink_t, kr_t)
            project_rope(wqT, xq_t, SQ, NQ, cosq_t, sinq_t, qr_t)

        # ================= Phase B: attention =================
        with ExitStack() as bctx:
            exp_pool = bctx.enter_context(tc.tile_pool(name="exp", bufs=2))
            rc_pool = bctx.enter_context(tc.tile_pool(name="rc", bufs=2))
            psS = bctx.enter_context(
                tc.tile_pool(name="psS", bufs=4, space="PSUM"))
            psC = bctx.enter_context(
                tc.tile_pool(name="psC", bufs=2, space="PSUM"))
            psB = bctx.enter_context(
                tc.tile_pool(name="psB", bufs=2, space="PSUM"))

            for h in range(H):
                ct = h // 2
                b0 = (h % 2) * 64
                kr = kr_t[ct]
                qr = qr_t[ct]
                expt = [exp_pool.tile([128, (KT_S // 2) * SQ], BF, tag="exp",
                                      name="exp")
                        for _ in range(2)]  # [kt parity][... ] see below
                # scores + exp for all 16 k-tiles x 2 q-chunks
                # expt layout: per k-tile a [128, 512] block; store the 16
                # k-tiles as two tiles of [128, 8*512] (kt%2 picks tile,
                # kt//2 picks block) purely to keep tile sizes uniform.
                for kt in range(KT_S):
                    et = expt[kt % 2]
                    blk = kt // 2
                    for qb in range(NQ):
                        ps = psS.tile([128, 512], F32, tag="psS")
                        nc.tensor.matmul(
                            ps[:],
                            kr[b0:b0 + 64, kt * 128:(kt + 1) * 128],
                            qr[b0:b0 + 64, qb * 512:(qb + 1) * 512],
                            start=True, stop=True)
                        nc.scalar.activation(
                            et[:, blk * 1024 + qb * 512:
                               blk * 1024 + (qb + 1) * 512],
                            ps[:], mybir.ActivationFunctionType.Exp,
                            scale=0.125)
                for qb in range(NQ):
                    cps = psC.tile([65, 512], F32, tag="psC")
                    for kt in range(KT_S):
                        et = expt[kt % 2]
                        blk = kt // 2
                        nc.tensor.matmul(
                            cps[:],
                            v_t[kt][:, h * (DH + 1):(h + 1) * (DH + 1)],
                            et[:, blk * 1024 + qb * 512:
                               blk * 1024 + (qb + 1) * 512],
                            start=(kt == 0), stop=(kt == KT_S - 1))
                    rc = rc_pool.tile([65, 512], BF, tag="rc")
                    with nc.allow_low_precision(reason="bf16 softmax denom"):
                        nc.vector.reciprocal(rc[64:65, :], cps[64:65, :])
                    bc = psB.tile([64, 512], F32, tag="psB")
                    nc.tensor.matmul(
                        bc[:], ones_t[64:65, 0:DH], rc[64:65, :],
                        start=True, stop=True)
                    bcs = rc_pool.tile([64, 512], F32, tag="bcs", name="bcs")
                    nc.scalar.activation(bcs[:], bc[:],
                                         mybir.ActivationFunctionType.Copy)
                    nc.vector.tensor_mul(
                        ctxT_t[ct][b0:b0 + 64, qb * 512:(qb + 1) * 512],
                        cps[0:64, :], bcs[:])

        # ================= Phase C: output projection =================
        with ExitStack() as cctx:
            osb_pool = cctx.enter_context(tc.tile_pool(name="osb", bufs=3))
            psO = cctx.enter_context(
                tc.tile_pool(name="psO", bufs=4, space="PSUM"))
            for m in range(KT_SQ):
                ot = osb_pool.tile([128, D], F32, tag="osb")
                for n in range(NO):
                    ps = psO.tile([128, 512], F32, tag="psO")
                    for k in range(KT_D):
                        nc.tensor.matmul(
                            ps[:], ctxT_t[k][:, m * 128:(m + 1) * 128],
                            wo_t[k][:, n * 512:(n + 1) * 512],
                            start=(k == 0), stop=(k == KT_D - 1))
                    nc.scalar.activation(ot[:, n * 512:(n + 1) * 512], ps[:],
                                         mybir.ActivationFunctionType.Copy)
                nc.sync.dma_start(out[m * 128:(m + 1) * 128, :], ot[:])

    nc.compile()
    return nc


_NC = None
LAST_RESULT = None
LAST_IN_MAPS = None


def _get_nc():
    global _NC
    if _NC is None:
        _NC = _build()
    return _NC


def kernel(x, cos, sin, wq, wk, wv, wo):
    global LAST_RESULT, LAST_IN_MAPS
    x = np.asarray(x)
    cos = np.asarray(cos)
    sin = np.asarray(sin)

    def bf(a):
        return np.ascontiguousarray(a, dtype=np.float32).astype(BF_NP)

    cosT = cos[0, :, 0, :].T.astype(np.float32)   # [DH, S]
    sinT = sin[0, :, 0, :].T.astype(np.float32)
    sinr = np.concatenate([-sinT[:DH // 2], sinT[DH // 2:]], axis=0)
    cos2 = np.concatenate([cosT, cosT], axis=0)   # [128, S]
    sinr2 = np.concatenate([sinr, sinr], axis=0)

    wqT, wkT, wvT, woT = (bf(w.T) for w in (wq, wk, wv, wo))
    in_maps = []
    for c in range(NCORES):
        b, half = c // 2, c % 2
        q0 = half * SQ
        xTb = bf(x[b].T)
        in_maps.append({
            "xT": xTb,
            "xqT": np.ascontiguousarray(xTb[:, q0:q0 + SQ]),
            "wqT": wqT, "wkT": wkT, "wvT": wvT, "woT": woT,
            "cosq": bf(cos2[:, q0:q0 + SQ]),
            "sinrq": bf(sinr2[:, q0:q0 + SQ]),
            "cosk": bf(cos2), "sinrk": bf(sinr2),
        })

    LAST_IN_MAPS = in_maps
    nc = _get_nc()
    res = bass_utils.run_bass_kernel_spmd(nc, in_maps,
                                          core_ids=list(range(NCORES)))
    LAST_RESULT = res
    out_full = np.empty((B, S, D), np.float32)
    for c in range(NCORES):
        b, half = c // 2, c % 2
        out_full[b, half * SQ:(half + 1) * SQ, :] = res.results[c]["out"]
    return out_full
